# revision 25
# baseline (speedup 1.0000x reference)
"""Deep-MMD loss kernel for Trainium2, sharded across 8 NeuronCores.

Strategy v3 (symmetric single-gram): the three gram matrices k_x, k_y, k_xy
are blocks of one symmetric 8192x8192 gram K over v=[x;y] with a common
kernel exp(-d_feat/sph - d_org/sq).  Split v into 16 panels of 512; core c
owns panels c (x-side) and c+8 (y-side).  Each unordered panel pair is
computed ONCE (pairs {a, a+d mod 16}, d=1..7 by the owner of panel a; d=8
pairs and both diagonals by core c).  Per core that is a uniform 17-block
program: x-rows over column windows 0..7, y-rows over windows 8..15, plus
x-rows over window 8 (the k_xy diagonal block).  The host places panels
into windows (any order), so the compiled program is identical on all 8
cores (SPMD).

Window ordering is chosen side-pure: within each pass, supertile pairs
(2t, 2t+1) hold panels of the same v-side except possibly supertile 0
(when the side counts have odd parity).  Pure tiles need only the per-tile
f32 row-sum total (the ACT accumulator sums internal f32 regardless of
output dtype); only supertile-0 tiles get a DVE half-reduce to split the
pair.  exp output kt is fp16: it feeds the column-sum matmuls directly
(no casts) and the variance-only row-vector split; all mmd2-relevant sums
stay f32.  Diagonals are re-exp'd to f32 from the same psum (bit-identical
to what the accumulator summed) so sum-trace cancels exactly.

All per-sample transforms (3-layer softplus MLP, W4 W4^T cholesky
v-transform, bf16 3-level splits, norm levels) are host-side f64 input
transforms; the device runs a pure streamed gram loop: 3 bf16 matmuls +
1 exp per 128x1024 chunk.  Validated on host at ~2.3e-4 rel err on mmd2
(vs 2e-2 tolerance).
"""

import numpy as np

N = 4096            # samples per side
NP = 16             # 512-sample panels over v = [x; y]
PW = 512            # panel width
NQ = 8192           # per-core q/ur columns (16 windows of 512)
NCORES = 8
HID = 10
NACC = 36           # per-tile f32 rowsum totals: x/y tiles 0..31, block16 32..35
NRSH = 8            # supertile-0 second-half rowsums (s*4 + ch)
NCS = 15            # column-sum window slots


def _build_bass(ascale):
    import concourse.bass as bass  # noqa: F401
    import concourse.mybir as mybir
    import concourse.tile as tile
    from concourse import bacc

    f32 = mybir.dt.float32
    f16 = mybir.dt.float16
    f8 = mybir.dt.float8e4
    bf16 = mybir.dt.bfloat16
    AFT = mybir.ActivationFunctionType
    ALU = mybir.AluOpType
    DR = mybir.MatmulPerfMode.DoubleRow

    nc = bacc.Bacc("TRN2")

    q8d = nc.dram_tensor("q8d", [128, 2, NQ], f8, kind="ExternalInput")
    qs8d = nc.dram_tensor("qs8d", [128, 2, 1024], f8, kind="ExternalInput")
    uld = nc.dram_tensor("uld", [128, 1024], bf16, kind="ExternalInput")
    urd = nc.dram_tensor("urd", [128, NQ], bf16, kind="ExternalInput")
    eyed = nc.dram_tensor("eyed", [128, 128], f32, kind="ExternalInput")
    ebd = nc.dram_tensor("ebd", [128, 1024], bf16, kind="ExternalInput")

    outd = nc.dram_tensor("outd", [128, NACC + NRSH + 4], f32,
                          kind="ExternalOutput")
    csd = nc.dram_tensor("csd", [1, NCS * PW], f32, kind="ExternalOutput")

    def cs_slot(w):
        # x-pass windows 1..7 -> 0..6; y-pass 9..15 -> 7..13; block16 -> 14
        return (w - 1) if w < 8 else 7 + (w - 9)

    with tile.TileContext(nc) as tc:
        with tc.tile_pool(name="persist", bufs=1) as pp:
            t_q8 = pp.tile([128, 2, NQ], f8, name="q8", tag="q8")
            t_qs8 = pp.tile([128, 2, 1024], f8, name="qs8", tag="qs8")
            t_ul = pp.tile([128, 1024], bf16, name="ul", tag="ul")
            t_ur = pp.tile([128, NQ], bf16, name="ur", tag="ur")
            t_eye = pp.tile([128, 128], f32, name="eye", tag="eye")
            t_ones = pp.tile([128, 1], bf16, name="ones1", tag="ones1")
            t_out = pp.tile([128, NACC + NRSH + 4], f32, name="out",
                            tag="out")
            t_acc = t_out[:, 0:NACC]
            t_rsh = t_out[:, NACC:NACC + NRSH]
            t_dg = t_out[:, NACC + NRSH:NACC + NRSH + 4]
            t_eb = pp.tile([128, 1024], bf16, name="eb", tag="eb")
            t_cs = pp.tile([1, NCS * PW], f32, name="cs", tag="cs")
            t_warm = pp.tile([128, 16], f32, name="warm", tag="warm")

            # input loads: first-needed operands first (sync), q on gpsimd
            nc.sync.dma_start(t_qs8[:], qs8d[:])
            nc.sync.dma_start(t_ul[:], uld[:])
            nc.sync.dma_start(t_ur[:, 0:512], urd[:, 0:512])
            nc.sync.dma_start(t_ur[:, 512:1024], urd[:, 512:1024])
            nc.sync.dma_start(t_eb[:], ebd[:])
            nc.sync.dma_start(t_eye[:], eyed[:])
            nc.gpsimd.dma_start(t_q8[:, :, 0:512], q8d[:, :, 0:512])
            nc.gpsimd.dma_start(t_q8[:, :, 512:1024], q8d[:, :, 512:1024])
            for k in range(1, 8):
                sl = slice(k * 1024, (k + 1) * 1024)
                nc.gpsimd.dma_start(t_q8[:, :, sl], q8d[:, :, sl])
                nc.sync.dma_start(t_ur[:, sl], urd[:, sl])
            # trigger the exp table load immediately (off the data path)
            nc.vector.memset(t_warm[:], 0.0)
            nc.scalar.activation(t_warm[:], t_warm[:], AFT.Exp)
            nc.vector.memset(t_ones[:], 1.0)

            with tc.tile_pool(name="kp", bufs=6) as kp, \
                 tc.tile_pool(name="dp", bufs=3) as dp, \
                 tc.tile_pool(name="gps", bufs=3, space="PSUM") as gps, \
                 tc.tile_pool(name="cps", bufs=1, space="PSUM") as cps:

                def gram_chunk(pk, qoff, w, bump=None):
                    ns = slice(w * PW, (w + 1) * PW)
                    nc.tensor.matmul(pk, t_qs8[:, :, qoff], t_q8[:, :, ns],
                                     start=True, stop=False, perf_mode=DR)
                    if bump is not None:
                        # add VAL to the diagonal (cols 128*ch..) so the
                        # diag entries underflow to exactly 0 after exp
                        nc.tensor.matmul(
                            pk, t_eb[:, 896:1024],
                            t_eb[:, 384 - 128 * bump:896 - 128 * bump],
                            start=False, stop=False)
                    nc.tensor.matmul(pk, t_ul[:, qoff], t_ur[:, ns],
                                     start=False, stop=True)

                def diag(pk, ch, dcol):
                    # f32 re-exp of the diagonal 128x128 from the same psum:
                    # bit-identical to what the accumulator summed, so the
                    # host-side sum-trace cancellation is exact.
                    dkt = dp.tile([128, 128], f32, name="dkt", tag="dkt")
                    nc.scalar.activation(dkt[:], pk[:, ch * 128:(ch + 1) * 128],
                                         AFT.Exp, scale=ascale)
                    dtmp = dp.tile([128, 128], f32, name="dtmp", tag="dtmp")
                    nc.vector.scalar_tensor_tensor(
                        dtmp[:], dkt[:], 1.0, t_eye[:],
                        ALU.mult, ALU.mult, accum_out=dcol)

                # colsum matmuls are emitted one chunk-iteration late so the
                # in-order PE queue never waits on the producing ACTIVATE
                pending = []

                def flush(n):
                    while len(pending) > n:
                        pending.pop(0)()

                def tile_work(s, t, ch, csp0, csp1, w0, w1, cs_copy=True):
                    ti = (s * 16 + t * 4 + ch) if s >= 0 else 32 + ch
                    qoff = slice(max(s, 0) * 512 + ch * 128,
                                 max(s, 0) * 512 + (ch + 1) * 128)
                    pk = gps.tile([128, 1024], f32, name="pk", tag="pk")
                    gram_chunk(pk[:, 0:512], qoff, w0,
                               bump=ch if (s >= 0 and t == 0) else None)
                    if w1 is not None:
                        gram_chunk(pk[:, 512:1024], qoff, w1)
                    kt = kp.tile([128, 1024], f16, name="kt", tag="kt")
                    if w1 is not None:
                        nc.scalar.activation(kt[:], pk[:], AFT.Exp, scale=ascale,
                                             accum_out=t_acc[:, ti:ti + 1])
                    else:
                        nc.scalar.activation(kt[:, 0:512], pk[:, 0:512],
                                             AFT.Exp, scale=ascale,
                                             accum_out=t_acc[:, ti:ti + 1])
                    if s < 0:
                        diag(pk, ch, t_dg[:, ch:ch + 1])
                    if s >= 0 and t == 0:
                        nc.vector.reduce_sum(
                            t_rsh[:, s * 4 + ch:s * 4 + ch + 1],
                            kt[:, 512:1024], axis=mybir.AxisListType.X)

                    def emit_cs():
                        if csp0 is not None:
                            nc.tensor.matmul(csp0[:], t_ones[:], kt[:, 0:512],
                                             start=(ch == 0), stop=(ch == 3))
                        if csp1 is not None:
                            nc.tensor.matmul(csp1[:], t_ones[:],
                                             kt[:, 512:1024],
                                             start=(ch == 0), stop=(ch == 3))
                        if ch == 3 and cs_copy:
                            for csp, w in ((csp0, w0), (csp1, w1)):
                                if csp is not None:
                                    sl = cs_slot(w)
                                    nc.vector.tensor_copy(
                                        t_cs[0:1, sl * PW:(sl + 1) * PW],
                                        csp[:])
                    pending.append(emit_cs)
                    flush(1)

                for s in range(2):
                    for t in range(4):
                        w0, w1 = 8 * s + 2 * t, 8 * s + 2 * t + 1
                        csp0 = (cps.tile([1, PW], f32, name="csp0", tag="csp0")
                                if t > 0 else None)
                        csp1 = cps.tile([1, PW], f32, name="csp1", tag="csp1")
                        for ch in range(4):
                            tile_work(s, t, ch, csp0, csp1, w0, w1)

                # block16: x-rows over window 8 (the k_xy diagonal block)
                cspb = cps.tile([1, PW], f32, name="csp0", tag="csp0")
                for ch in range(4):
                    tile_work(-1, 0, ch, cspb, None, 8, None, cs_copy=False)
                flush(0)
                nc.vector.tensor_copy(t_cs[0:1, 14 * PW:15 * PW], cspb[:])
                nc.sync.dma_start(outd[:], t_out[:])
                nc.sync.dma_start(csd[:], t_cs[:])

    nc.compile()
    return nc


_NC_CACHE = None
_LAST_RESULT = None


def _harden_tracing():
    """Make run_bass_kernel_spmd(trace=True / BASS_TRACE=1) survive in
    containers whose antenv package lacks axon_hooks, and whose bucket
    upload is unavailable. No-ops when everything is present."""
    import sys
    import types
    try:
        import antenv.axon_hooks  # noqa: F401
    except ImportError:
        mod = types.ModuleType("antenv.axon_hooks")
        mod._hook = None
        mod.set_axon_ntff_profile_hook = lambda h: setattr(mod, "_hook", h)
        mod.get_axon_ntff_profile_hook = lambda: mod._hook
        sys.modules["antenv.axon_hooks"] = mod
        try:
            import antenv
            antenv.axon_hooks = mod
        except ImportError:
            pass
        try:
            from trn_agent_boot.trn_boot import _ntff_profile_via_ctypes
            hook = _ntff_profile_via_ctypes("/opt/axon/libaxon_pjrt.so")
            if hook is not None:
                mod.set_axon_ntff_profile_hook(hook)
        except Exception:
            pass
    from concourse import bass_utils
    if not getattr(bass_utils.upload_artifacts, "_mmd_safe", False):
        orig = bass_utils.upload_artifacts

        def safe_upload(tmpdir):
            try:
                return orig(tmpdir)
            except Exception:
                return tmpdir

        safe_upload._mmd_safe = True
        bass_utils.upload_artifacts = safe_upload


def _softplus(x):
    return np.log1p(np.exp(-np.abs(x))) + np.maximum(x, 0)


def _window_maps():
    """Per-core window -> panel assignment, side-pure except supertile 0."""
    wmaps = []
    for c in range(NCORES):
        px = [(c + d) % NP for d in range(1, 8)]
        Px = [p for p in px if p < 8]
        Py = [p for p in px if p >= 8]
        w1 = Px.pop(0) if len(Px) % 2 == 1 else Py.pop(0)
        xw = [c, w1] + Px + Py
        py = [(c + 8 + d) % NP for d in range(1, 8)]
        Qy = [p for p in py if p >= 8]
        Qx = [p for p in py if p < 8]
        w9 = Qy.pop(0) if len(Qy) % 2 == 1 else Qx.pop(0)
        yw = [c + 8, w9] + Qy + Qx
        wmaps.append(xw + yw)
    return wmaps


def _host_prep(X, Y, W1, b1, W2, b2, W3, b3, W4, b4,
               epsilon_opt, sigma_q_opt, sigma_phi_opt):
    """f64 input transforms -> per-core input maps (window-permuted)."""
    import ml_dtypes
    bfd = ml_dtypes.bfloat16

    X = np.asarray(X, np.float64)
    Y = np.asarray(Y, np.float64)
    W1 = np.asarray(W1, np.float64)
    W2 = np.asarray(W2, np.float64)
    W3 = np.asarray(W3, np.float64)
    W4 = np.asarray(W4, np.float64)
    b1 = np.asarray(b1, np.float64)
    b2 = np.asarray(b2, np.float64)
    b3 = np.asarray(b3, np.float64)
    sq = float(np.asarray(sigma_q_opt, np.float64) ** 2)
    sph = float(np.asarray(sigma_phi_opt, np.float64) ** 2)
    # eps ~ 5e-11 mixture term contributes ~3e-16 to mmd2; dropped (b4
    # cancels exactly inside pairwise feature distances).

    fo = np.concatenate([X, Y], 0)                  # v-space [8192, 256]
    h = _softplus(fo @ W1 + b1)
    h = _softplus(h @ W2 + b2)
    h = _softplus(h @ W3 + b3)
    G = W4 @ W4.T
    lv = np.sqrt(2.0 / sph) * np.linalg.cholesky(G)
    u = h @ lv
    u = u - u.mean(0)                               # d_feat/sph = |ui-uj|^2/2

    # the device psum is P = SC*dm so the raw-fp8 org cross term -x.y rides
    # with coefficient 1; the exp applies scale = -1/SC = -2/sq.
    SC = sq / 2.0
    u = u * np.sqrt(SC)

    def lvl(a):
        hi = a.astype(bfd).astype(np.float64)
        return hi, a - hi

    uh, r = lvl(u)
    um, r = lvl(r)
    ulo, _ = lvl(r)

    f8 = ml_dtypes.float8_e4m3
    x8 = fo.astype(f8)                              # org side fp8
    xq = x8.astype(np.float64)
    xn = 0.5 * (xq * xq).sum(1)                     # = SC * |x8|^2 / sq
    s1 = (0.5 * u * u).astype(bfd).astype(np.float64)   # per-comp self hi
    a_exact = 0.5 * (u * u).sum(1) + xn
    rem = a_exact - s1.sum(1)
    r1, rr = lvl(rem)
    r2, rr = lvl(rr)
    r3, _ = lvl(rr)

    # global UL / UR row content  (psum = a_i + a_j - u_i.u_j - 2/sq x_i.x_j;
    # AA products interleaved with self-squares keep f32 partials bounded)
    M = 2 * N
    ulg = np.zeros((128, M), bfd)
    urg = np.zeros((128, M), bfd)
    for k in range(HID):
        ulg[3 * k] = s1[:, k]
        urg[3 * k] = 1.0
        ulg[3 * k + 1] = -uh[:, k]
        urg[3 * k + 1] = uh[:, k]
        ulg[3 * k + 2] = 1.0
        urg[3 * k + 2] = s1[:, k]
    ulg[30:40] = -um.T
    urg[30:40] = uh.T
    ulg[40:50] = -uh.T
    urg[40:50] = um.T
    ulg[50:60] = -um.T
    urg[50:60] = um.T
    ulg[60:70] = -ulo.T
    urg[60:70] = uh.T
    ulg[70:80] = -uh.T
    urg[70:80] = ulo.T
    for i, rv in enumerate((r1, r2, r3)):
        ulg[80 + i] = rv
        urg[80 + i] = 1.0
        ulg[83 + i] = 1.0
        urg[83 + i] = rv

    # fp8 org operands: [128, 2, cols] with contraction dims (k*128+p)
    orgT = x8.T.reshape(2, 128, 2 * N).transpose(1, 0, 2)    # [128, 2, 8192]
    qsT = (-x8).T.reshape(2, 128, 2 * N).transpose(1, 0, 2)  # negated lhsT

    common_eye = np.eye(128, dtype=np.float32)
    eb = np.zeros((128, 1024), bfd)
    for p in range(128):
        eb[p, 384 + p] = 262144.0      # diag-kill bump value (2^18)
        eb[p, 896 + p] = 1.0           # identity lhsT for the bump matmul
    wmaps = _window_maps()
    in_maps = []
    for c in range(NCORES):
        panels = np.array(wmaps[c])
        P = (panels[:, None] * PW + np.arange(PW)[None, :]).reshape(-1)
        own = np.concatenate([np.arange(c * PW, (c + 1) * PW),
                              np.arange(N + c * PW, N + (c + 1) * PW)])
        m = {
            "q8d": np.ascontiguousarray(orgT[:, :, P]),
            "qs8d": np.ascontiguousarray(qsT[:, :, own]),
            "uld": np.ascontiguousarray(ulg[:, own]),
            "urd": np.ascontiguousarray(urg[:, P]),
            "eyed": common_eye,
            "ebd": eb,
        }
        in_maps.append(m)
    return in_maps, wmaps, float(-1.0 / SC)


def _emulate_core(m, ascale):
    """Pure-numpy emulation of the device program for one core's inputs.
    Mirrors the bass loop (f64 stand-in for f32; fp16 where the device
    rounds kt)."""
    import ml_dtypes
    f16 = np.float16
    q = m["q8d"].astype(np.float64).transpose(1, 0, 2).reshape(256, NQ)
    qs = m["qs8d"].astype(np.float64).transpose(1, 0, 2).reshape(256, 1024)
    ul = m["uld"].astype(np.float64)                           # [128, 1024]
    ur = m["urd"].astype(np.float64)                           # [128, 8192]
    acc = np.zeros((128, NACC))
    rsh = np.zeros((128, NRSH))
    dg = np.zeros((128, 4))
    cs = np.zeros((1, NCS * PW))

    def window_chunk(qoff, w):
        ns = slice(w * PW, (w + 1) * PW)
        pk = qs[:, qoff].T @ q[:, ns] + ul[:, qoff].T @ ur[:, ns]
        return np.exp(ascale * pk)

    def cs_slot(w):
        return (w - 1) if w < 8 else 7 + (w - 9)

    for s in range(2):
        for t in range(4):
            w0, w1 = 8 * s + 2 * t, 8 * s + 2 * t + 1
            c0 = np.zeros(PW)
            c1 = np.zeros(PW)
            for ch in range(4):
                ti = s * 16 + t * 4 + ch
                qoff = slice(s * 512 + ch * 128, s * 512 + (ch + 1) * 128)
                k0 = window_chunk(qoff, w0)
                k1 = window_chunk(qoff, w1)
                if t == 0:
                    dsl = k0[:, ch * 128:(ch + 1) * 128]
                    dsl[np.arange(128), np.arange(128)] = 0.0
                acc[:, ti] = k0.sum(1) + k1.sum(1)
                k0h = k0.astype(f16).astype(np.float64)
                k1h = k1.astype(f16).astype(np.float64)
                if t == 0:
                    rsh[:, s * 4 + ch] = k1h.sum(1)
                if t > 0:
                    c0 += k0h.sum(0)
                c1 += k1h.sum(0)
            if t > 0:
                cs[0, cs_slot(w0) * PW:(cs_slot(w0) + 1) * PW] = c0
            cs[0, cs_slot(w1) * PW:(cs_slot(w1) + 1) * PW] = c1
    cb = np.zeros(PW)
    for ch in range(4):
        qoff = slice(ch * 128, (ch + 1) * 128)
        k0 = window_chunk(qoff, 8)
        acc[:, 32 + ch] = k0.sum(1)
        cb += k0.astype(f16).astype(np.float64).sum(0)
        dg[:, ch] = np.diag(k0[:, ch * 128:(ch + 1) * 128])
    cs[0, 14 * PW:15 * PW] = cb
    return {"outd": np.concatenate([acc, rsh, dg], 1), "csd": cs}


def _reduce(results, wmaps):
    """f64 host-side assembly of mmd2/var from per-core outputs."""
    rs_x = np.zeros(N)
    rs_y = np.zeros(N)
    rs_xy = np.zeros(N)
    cs_xy = np.zeros(N)
    sums = {"x": 0.0, "y": 0.0, "z": 0.0}
    dgs = {"x": 0.0, "y": 0.0, "z": 0.0}

    for c in range(NCORES):
        out = results[c]
        full = out["outd"].astype(np.float64)
        acc = full[:, 0:NACC]
        rsh = full[:, NACC:NACC + NRSH]
        dg = full[:, NACC + NRSH:NACC + NRSH + 4]
        cs = out["csd"].astype(np.float64)[0]
        wm = wmaps[c]

        def route_rows(v, s, pw, wt):
            # rowsum contribution of rows (side s, own panel) over panel pw
            rows = c * PW + np.arange(128) + route_rows.choff
            S = v.sum()
            if s == 0 and pw < 8:
                rs_x[rows] += v
                sums["x"] += S * wt
            elif s == 0:
                rs_xy[rows] += v
                sums["z"] += S
            elif pw >= 8:
                rs_y[rows] += v
                sums["y"] += S * wt
            else:
                cs_xy[rows] += v
                sums["z"] += S

        for s in range(2):
            for t in range(4):
                pw0, pw1 = wm[8 * s + 2 * t], wm[8 * s + 2 * t + 1]
                for ch in range(4):
                    ti = s * 16 + t * 4 + ch
                    route_rows.choff = ch * 128
                    T = acc[:, ti]
                    if t == 0:
                        h2 = rsh[:, s * 4 + ch]
                        route_rows(T - h2, s, pw0, 1.0)   # diag window
                        route_rows(h2, s, pw1, 2.0)
                    else:
                        # side-pure pair: both windows same bucket
                        route_rows(T, s, pw0, 2.0)
        # block16: x-rows over window 8 (y-panel c)
        for ch in range(4):
            rows = c * PW + ch * 128 + np.arange(128)
            v = acc[:, 32 + ch]
            rs_xy[rows] += v
            sums["z"] += v.sum()

        # column sums (contribute to the transposed image's rows)
        def cs_cols(pw):
            base = pw * PW if pw < 8 else (pw - 8) * PW
            return base + np.arange(PW)

        for s in range(2):
            wlist = range(1, 8) if s == 0 else range(9, 16)
            for w in wlist:
                slot = (w - 1) if w < 8 else 7 + (w - 9)
                v = cs[slot * PW:(slot + 1) * PW]
                pw = wm[w]
                cols = cs_cols(pw)
                if s == 0 and pw < 8:
                    rs_x[cols] += v
                elif s == 0:
                    cs_xy[cols] += v
                elif pw >= 8:
                    rs_y[cols] += v
                else:
                    rs_xy[cols] += v
        cs_xy[cs_cols(wm[8])] += cs[14 * PW:15 * PW]

        dgs["z"] += dg[:, 0:4].sum()

    nn1 = float(N) * (N - 1)
    xx = (sums["x"] - dgs["x"]) / nn1
    yy = (sums["y"] - dgs["y"]) / nn1
    xy = (sums["z"] - dgs["z"]) / nn1
    mmd2 = xx - 2.0 * xy + yy

    # x/y gram diagonals were zeroed in-psum; their true value is 1
    hs = rs_x + rs_y - rs_xy - cs_xy + 2.0
    sum_h = (sums["x"] + N) + (sums["y"] + N) - 2.0 * sums["z"]
    v1 = (4.0 / N ** 3) * float(hs @ hs)
    v2 = (4.0 / N ** 4) * sum_h ** 2
    var = v1 - v2 + 1e-8
    return np.array([mmd2, var], np.float32)


def kernel(X, Y, W1, b1, W2, b2, W3, b3, W4, b4,
           epsilon_opt, sigma_q_opt, sigma_phi_opt, _emulate=False):
    global _NC_CACHE, _LAST_RESULT
    in_maps, wmaps, ascale = _host_prep(X, Y, W1, b1, W2, b2, W3, b3, W4, b4,
                                        epsilon_opt, sigma_q_opt, sigma_phi_opt)
    if _emulate:
        results = [_emulate_core(m, ascale) for m in in_maps]
        return _reduce(results, wmaps)

    from concourse import bass_utils
    _harden_tracing()
    if _NC_CACHE is None:
        _NC_CACHE = _build_bass(ascale)
    nc = _NC_CACHE
    res = bass_utils.run_bass_kernel_spmd(nc, in_maps,
                                          core_ids=list(range(NCORES)))
    _LAST_RESULT = res
    return _reduce(res.results, wmaps)


# revision 26
# speedup vs baseline: 1.0531x; 1.0531x over previous
"""Deep-MMD loss kernel for Trainium2, sharded across 8 NeuronCores.

Strategy v3 (symmetric single-gram): the three gram matrices k_x, k_y, k_xy
are blocks of one symmetric 8192x8192 gram K over v=[x;y] with a common
kernel exp(-d_feat/sph - d_org/sq).  Split v into 16 panels of 512; core c
owns panels c (x-side) and c+8 (y-side).  Each unordered panel pair is
computed ONCE (pairs {a, a+d mod 16}, d=1..7 by the owner of panel a; d=8
pairs and both diagonals by core c).  Per core that is a uniform 17-block
program: x-rows over column windows 0..7, y-rows over windows 8..15, plus
x-rows over window 8 (the k_xy diagonal block).  The host places panels
into windows (any order), so the compiled program is identical on all 8
cores (SPMD).

Window ordering is chosen side-pure: within each pass, supertile pairs
(2t, 2t+1) hold panels of the same v-side except possibly supertile 0
(when the side counts have odd parity).  Pure tiles need only the per-tile
f32 row-sum total (the ACT accumulator sums internal f32 regardless of
output dtype); only supertile-0 tiles get a DVE half-reduce to split the
pair.  exp output kt is fp16: it feeds the column-sum matmuls directly
(no casts) and the variance-only row-vector split; all mmd2-relevant sums
stay f32.  Diagonals are re-exp'd to f32 from the same psum (bit-identical
to what the accumulator summed) so sum-trace cancels exactly.

All per-sample transforms (3-layer softplus MLP, W4 W4^T cholesky
v-transform, bf16 3-level splits, norm levels) are host-side f64 input
transforms; the device runs a pure streamed gram loop: 3 bf16 matmuls +
1 exp per 128x1024 chunk.  Validated on host at ~2.3e-4 rel err on mmd2
(vs 2e-2 tolerance).
"""

import numpy as np

N = 4096            # samples per side
NP = 16             # 512-sample panels over v = [x; y]
PW = 512            # panel width
NQ = 8192           # per-core q/ur columns (16 windows of 512)
NCORES = 8
HID = 10
NACC = 36           # per-tile f32 rowsum totals: x/y tiles 0..31, block16 32..35
NRSH = 8            # supertile-0 second-half rowsums (s*4 + ch)
NCS = 15            # column-sum window slots


def _build_bass(ascale):
    import concourse.bass as bass  # noqa: F401
    import concourse.mybir as mybir
    import concourse.tile as tile
    from concourse import bacc

    f32 = mybir.dt.float32
    f16 = mybir.dt.float16
    f8 = mybir.dt.float8e4
    bf16 = mybir.dt.bfloat16
    AFT = mybir.ActivationFunctionType
    ALU = mybir.AluOpType
    DR = mybir.MatmulPerfMode.DoubleRow

    nc = bacc.Bacc("TRN2")

    q8d = nc.dram_tensor("q8d", [128, 2, NQ], f8, kind="ExternalInput")
    qs8d = nc.dram_tensor("qs8d", [128, 2, 1024], f8, kind="ExternalInput")
    uld = nc.dram_tensor("uld", [128, 1024], bf16, kind="ExternalInput")
    urd = nc.dram_tensor("urd", [128, NQ], bf16, kind="ExternalInput")
    eyed = nc.dram_tensor("eyed", [128, 128], f32, kind="ExternalInput")

    outd = nc.dram_tensor("outd", [128, NACC + NRSH + 12], f32,
                          kind="ExternalOutput")
    csd = nc.dram_tensor("csd", [1, NCS * PW], f32, kind="ExternalOutput")

    def cs_slot(w):
        # x-pass windows 1..7 -> 0..6; y-pass 9..15 -> 7..13; block16 -> 14
        return (w - 1) if w < 8 else 7 + (w - 9)

    with tile.TileContext(nc) as tc:
        with tc.tile_pool(name="persist", bufs=1) as pp:
            t_q8 = pp.tile([128, 2, NQ], f8, name="q8", tag="q8")
            t_qs8 = pp.tile([128, 2, 1024], f8, name="qs8", tag="qs8")
            t_ul = pp.tile([128, 1024], bf16, name="ul", tag="ul")
            t_ur = pp.tile([128, NQ], bf16, name="ur", tag="ur")
            t_eye = pp.tile([128, 128], f32, name="eye", tag="eye")
            t_ones = pp.tile([128, 1], bf16, name="ones1", tag="ones1")
            t_out = pp.tile([128, NACC + NRSH + 12], f32, name="out",
                            tag="out")
            t_acc = t_out[:, 0:NACC]
            t_rsh = t_out[:, NACC:NACC + NRSH]
            t_dg = t_out[:, NACC + NRSH:NACC + NRSH + 12]
            t_cs = pp.tile([1, NCS * PW], f32, name="cs", tag="cs")
            t_warm = pp.tile([128, 16], f32, name="warm", tag="warm")

            # input loads: first-needed operands first (sync), q on gpsimd
            nc.sync.dma_start(t_qs8[:], qs8d[:])
            nc.sync.dma_start(t_ul[:], uld[:])
            nc.sync.dma_start(t_ur[:, 0:512], urd[:, 0:512])
            nc.sync.dma_start(t_ur[:, 512:1024], urd[:, 512:1024])
            nc.sync.dma_start(t_eye[:], eyed[:])
            nc.gpsimd.dma_start(t_q8[:, :, 0:512], q8d[:, :, 0:512])
            nc.gpsimd.dma_start(t_q8[:, :, 512:1024], q8d[:, :, 512:1024])
            for k in range(1, 8):
                sl = slice(k * 1024, (k + 1) * 1024)
                nc.gpsimd.dma_start(t_q8[:, :, sl], q8d[:, :, sl])
                nc.sync.dma_start(t_ur[:, sl], urd[:, sl])
            # trigger the exp table load immediately (off the data path)
            nc.vector.memset(t_warm[:], 0.0)
            nc.scalar.activation(t_warm[:], t_warm[:], AFT.Exp)
            nc.vector.memset(t_ones[:], 1.0)

            with tc.tile_pool(name="kp", bufs=6) as kp, \
                 tc.tile_pool(name="dp", bufs=3) as dp, \
                 tc.tile_pool(name="gps", bufs=3, space="PSUM") as gps, \
                 tc.tile_pool(name="cps", bufs=1, space="PSUM") as cps:

                def gram_chunk(pk, qoff, w):
                    ns = slice(w * PW, (w + 1) * PW)
                    nc.tensor.matmul(pk, t_qs8[:, :, qoff], t_q8[:, :, ns],
                                     start=True, stop=False, perf_mode=DR)
                    nc.tensor.matmul(pk, t_ul[:, qoff], t_ur[:, ns],
                                     start=False, stop=True)

                def diag(pk, ch, dcol):
                    # f32 re-exp of the diagonal 128x128 from the same psum:
                    # bit-identical to what the accumulator summed, so the
                    # host-side sum-trace cancellation is exact.
                    dkt = dp.tile([128, 128], f32, name="dkt", tag="dkt")
                    nc.scalar.activation(dkt[:], pk[:, ch * 128:(ch + 1) * 128],
                                         AFT.Exp, scale=ascale)
                    dtmp = dp.tile([128, 128], f32, name="dtmp", tag="dtmp")
                    nc.vector.scalar_tensor_tensor(
                        dtmp[:], dkt[:], 1.0, t_eye[:],
                        ALU.mult, ALU.mult, accum_out=dcol)

                # colsum matmuls are emitted one chunk-iteration late so the
                # in-order PE queue never waits on the producing ACTIVATE
                pending = []

                def flush(n):
                    while len(pending) > n:
                        pending.pop(0)()

                def tile_work(s, t, ch, csp0, csp1, w0, w1, cs_copy=True):
                    ti = (s * 16 + t * 4 + ch) if s >= 0 else 32 + ch
                    qoff = slice(max(s, 0) * 512 + ch * 128,
                                 max(s, 0) * 512 + (ch + 1) * 128)
                    pk = gps.tile([128, 1024], f32, name="pk", tag="pk")
                    gram_chunk(pk[:, 0:512], qoff, w0)
                    if w1 is not None:
                        gram_chunk(pk[:, 512:1024], qoff, w1)
                    kt = kp.tile([128, 1024], f16, name="kt", tag="kt")
                    if w1 is not None:
                        nc.scalar.activation(kt[:], pk[:], AFT.Exp, scale=ascale,
                                             accum_out=t_acc[:, ti:ti + 1])
                    else:
                        nc.scalar.activation(kt[:, 0:512], pk[:, 0:512],
                                             AFT.Exp, scale=ascale,
                                             accum_out=t_acc[:, ti:ti + 1])
                    if (s >= 0 and t == 0) or s < 0:
                        dcol = 8 + ch if s < 0 else s * 4 + ch
                        diag(pk, ch, t_dg[:, dcol:dcol + 1])
                    if s >= 0 and t == 0:
                        nc.vector.reduce_sum(
                            t_rsh[:, s * 4 + ch:s * 4 + ch + 1],
                            kt[:, 512:1024], axis=mybir.AxisListType.X)

                    def emit_cs():
                        if csp0 is not None:
                            nc.tensor.matmul(csp0[:], t_ones[:], kt[:, 0:512],
                                             start=(ch == 0), stop=(ch == 3))
                        if csp1 is not None:
                            nc.tensor.matmul(csp1[:], t_ones[:],
                                             kt[:, 512:1024],
                                             start=(ch == 0), stop=(ch == 3))
                        if ch == 3 and cs_copy:
                            for csp, w in ((csp0, w0), (csp1, w1)):
                                if csp is not None:
                                    sl = cs_slot(w)
                                    nc.vector.tensor_copy(
                                        t_cs[0:1, sl * PW:(sl + 1) * PW],
                                        csp[:])
                    pending.append(emit_cs)
                    flush(1)

                for s in range(2):
                    for t in range(4):
                        w0, w1 = 8 * s + 2 * t, 8 * s + 2 * t + 1
                        csp0 = (cps.tile([1, PW], f32, name="csp0", tag="csp0")
                                if t > 0 else None)
                        csp1 = cps.tile([1, PW], f32, name="csp1", tag="csp1")
                        for ch in range(4):
                            tile_work(s, t, ch, csp0, csp1, w0, w1)

                # block16: x-rows over window 8 (the k_xy diagonal block)
                cspb = cps.tile([1, PW], f32, name="csp0", tag="csp0")
                for ch in range(4):
                    tile_work(-1, 0, ch, cspb, None, 8, None, cs_copy=False)
                flush(0)
                nc.vector.tensor_copy(t_cs[0:1, 14 * PW:15 * PW], cspb[:])
                nc.sync.dma_start(outd[:], t_out[:])
                nc.sync.dma_start(csd[:], t_cs[:])

    nc.compile()
    return nc


_NC_CACHE = None
_LAST_RESULT = None


def _harden_tracing():
    """Make run_bass_kernel_spmd(trace=True / BASS_TRACE=1) survive in
    containers whose antenv package lacks axon_hooks, and whose bucket
    upload is unavailable. No-ops when everything is present."""
    import sys
    import types
    try:
        import antenv.axon_hooks  # noqa: F401
    except ImportError:
        mod = types.ModuleType("antenv.axon_hooks")
        mod._hook = None
        mod.set_axon_ntff_profile_hook = lambda h: setattr(mod, "_hook", h)
        mod.get_axon_ntff_profile_hook = lambda: mod._hook
        sys.modules["antenv.axon_hooks"] = mod
        try:
            import antenv
            antenv.axon_hooks = mod
        except ImportError:
            pass
        try:
            from trn_agent_boot.trn_boot import _ntff_profile_via_ctypes
            hook = _ntff_profile_via_ctypes("/opt/axon/libaxon_pjrt.so")
            if hook is not None:
                mod.set_axon_ntff_profile_hook(hook)
        except Exception:
            pass
    from concourse import bass_utils
    if not getattr(bass_utils.upload_artifacts, "_mmd_safe", False):
        orig = bass_utils.upload_artifacts

        def safe_upload(tmpdir):
            try:
                return orig(tmpdir)
            except Exception:
                return tmpdir

        safe_upload._mmd_safe = True
        bass_utils.upload_artifacts = safe_upload


def _softplus(x):
    return np.log1p(np.exp(-np.abs(x))) + np.maximum(x, 0)


def _window_maps():
    """Per-core window -> panel assignment, side-pure except supertile 0."""
    wmaps = []
    for c in range(NCORES):
        px = [(c + d) % NP for d in range(1, 8)]
        Px = [p for p in px if p < 8]
        Py = [p for p in px if p >= 8]
        w1 = Px.pop(0) if len(Px) % 2 == 1 else Py.pop(0)
        xw = [c, w1] + Px + Py
        py = [(c + 8 + d) % NP for d in range(1, 8)]
        Qy = [p for p in py if p >= 8]
        Qx = [p for p in py if p < 8]
        w9 = Qy.pop(0) if len(Qy) % 2 == 1 else Qx.pop(0)
        yw = [c + 8, w9] + Qy + Qx
        wmaps.append(xw + yw)
    return wmaps


def _host_prep(X, Y, W1, b1, W2, b2, W3, b3, W4, b4,
               epsilon_opt, sigma_q_opt, sigma_phi_opt):
    """f64 input transforms -> per-core input maps (window-permuted)."""
    import ml_dtypes
    bfd = ml_dtypes.bfloat16

    X = np.asarray(X, np.float64)
    Y = np.asarray(Y, np.float64)
    W1 = np.asarray(W1, np.float64)
    W2 = np.asarray(W2, np.float64)
    W3 = np.asarray(W3, np.float64)
    W4 = np.asarray(W4, np.float64)
    b1 = np.asarray(b1, np.float64)
    b2 = np.asarray(b2, np.float64)
    b3 = np.asarray(b3, np.float64)
    sq = float(np.asarray(sigma_q_opt, np.float64) ** 2)
    sph = float(np.asarray(sigma_phi_opt, np.float64) ** 2)
    # eps ~ 5e-11 mixture term contributes ~3e-16 to mmd2; dropped (b4
    # cancels exactly inside pairwise feature distances).

    fo = np.concatenate([X, Y], 0)                  # v-space [8192, 256]
    h = _softplus(fo @ W1 + b1)
    h = _softplus(h @ W2 + b2)
    h = _softplus(h @ W3 + b3)
    G = W4 @ W4.T
    lv = np.sqrt(2.0 / sph) * np.linalg.cholesky(G)
    u = h @ lv
    u = u - u.mean(0)                               # d_feat/sph = |ui-uj|^2/2

    # the device psum is P = SC*dm so the raw-fp8 org cross term -x.y rides
    # with coefficient 1; the exp applies scale = -1/SC = -2/sq.
    SC = sq / 2.0
    u = u * np.sqrt(SC)

    def lvl(a):
        hi = a.astype(bfd).astype(np.float64)
        return hi, a - hi

    uh, r = lvl(u)
    um, r = lvl(r)
    ulo, _ = lvl(r)

    f8 = ml_dtypes.float8_e4m3
    x8 = fo.astype(f8)                              # org side fp8
    xq = x8.astype(np.float64)
    xn = 0.5 * (xq * xq).sum(1)                     # = SC * |x8|^2 / sq
    s1 = (0.5 * u * u).astype(bfd).astype(np.float64)   # per-comp self hi
    a_exact = 0.5 * (u * u).sum(1) + xn
    rem = a_exact - s1.sum(1)
    r1, rr = lvl(rem)
    r2, rr = lvl(rr)
    r3, _ = lvl(rr)

    # global UL / UR row content  (psum = a_i + a_j - u_i.u_j - 2/sq x_i.x_j;
    # AA products interleaved with self-squares keep f32 partials bounded)
    M = 2 * N
    ulg = np.zeros((128, M), bfd)
    urg = np.zeros((128, M), bfd)
    for k in range(HID):
        ulg[3 * k] = s1[:, k]
        urg[3 * k] = 1.0
        ulg[3 * k + 1] = -uh[:, k]
        urg[3 * k + 1] = uh[:, k]
        ulg[3 * k + 2] = 1.0
        urg[3 * k + 2] = s1[:, k]
    ulg[30:40] = -um.T
    urg[30:40] = uh.T
    ulg[40:50] = -uh.T
    urg[40:50] = um.T
    ulg[50:60] = -um.T
    urg[50:60] = um.T
    ulg[60:70] = -ulo.T
    urg[60:70] = uh.T
    ulg[70:80] = -uh.T
    urg[70:80] = ulo.T
    for i, rv in enumerate((r1, r2, r3)):
        ulg[80 + i] = rv
        urg[80 + i] = 1.0
        ulg[83 + i] = 1.0
        urg[83 + i] = rv

    # fp8 org operands: [128, 2, cols] with contraction dims (k*128+p)
    orgT = x8.T.reshape(2, 128, 2 * N).transpose(1, 0, 2)    # [128, 2, 8192]
    qsT = (-x8).T.reshape(2, 128, 2 * N).transpose(1, 0, 2)  # negated lhsT

    common_eye = np.eye(128, dtype=np.float32)
    wmaps = _window_maps()
    in_maps = []
    for c in range(NCORES):
        panels = np.array(wmaps[c])
        P = (panels[:, None] * PW + np.arange(PW)[None, :]).reshape(-1)
        own = np.concatenate([np.arange(c * PW, (c + 1) * PW),
                              np.arange(N + c * PW, N + (c + 1) * PW)])
        m = {
            "q8d": np.ascontiguousarray(orgT[:, :, P]),
            "qs8d": np.ascontiguousarray(qsT[:, :, own]),
            "uld": np.ascontiguousarray(ulg[:, own]),
            "urd": np.ascontiguousarray(urg[:, P]),
            "eyed": common_eye,
        }
        in_maps.append(m)
    return in_maps, wmaps, float(-1.0 / SC)


def _emulate_core(m, ascale):
    """Pure-numpy emulation of the device program for one core's inputs.
    Mirrors the bass loop (f64 stand-in for f32; fp16 where the device
    rounds kt)."""
    import ml_dtypes
    f16 = np.float16
    q = m["q8d"].astype(np.float64).transpose(1, 0, 2).reshape(256, NQ)
    qs = m["qs8d"].astype(np.float64).transpose(1, 0, 2).reshape(256, 1024)
    ul = m["uld"].astype(np.float64)                           # [128, 1024]
    ur = m["urd"].astype(np.float64)                           # [128, 8192]
    acc = np.zeros((128, NACC))
    rsh = np.zeros((128, NRSH))
    dg = np.zeros((128, 12))
    cs = np.zeros((1, NCS * PW))

    def window_chunk(qoff, w):
        ns = slice(w * PW, (w + 1) * PW)
        pk = qs[:, qoff].T @ q[:, ns] + ul[:, qoff].T @ ur[:, ns]
        return np.exp(ascale * pk)

    def cs_slot(w):
        return (w - 1) if w < 8 else 7 + (w - 9)

    for s in range(2):
        for t in range(4):
            w0, w1 = 8 * s + 2 * t, 8 * s + 2 * t + 1
            c0 = np.zeros(PW)
            c1 = np.zeros(PW)
            for ch in range(4):
                ti = s * 16 + t * 4 + ch
                qoff = slice(s * 512 + ch * 128, s * 512 + (ch + 1) * 128)
                k0 = window_chunk(qoff, w0)
                k1 = window_chunk(qoff, w1)
                acc[:, ti] = k0.sum(1) + k1.sum(1)
                k0h = k0.astype(f16).astype(np.float64)
                k1h = k1.astype(f16).astype(np.float64)
                if t == 0:
                    rsh[:, s * 4 + ch] = k1h.sum(1)
                    dg[:, s * 4 + ch] = np.diag(
                        k0[:, ch * 128:(ch + 1) * 128])
                if t > 0:
                    c0 += k0h.sum(0)
                c1 += k1h.sum(0)
            if t > 0:
                cs[0, cs_slot(w0) * PW:(cs_slot(w0) + 1) * PW] = c0
            cs[0, cs_slot(w1) * PW:(cs_slot(w1) + 1) * PW] = c1
    cb = np.zeros(PW)
    for ch in range(4):
        qoff = slice(ch * 128, (ch + 1) * 128)
        k0 = window_chunk(qoff, 8)
        acc[:, 32 + ch] = k0.sum(1)
        cb += k0.astype(f16).astype(np.float64).sum(0)
        dg[:, 8 + ch] = np.diag(k0[:, ch * 128:(ch + 1) * 128])
    cs[0, 14 * PW:15 * PW] = cb
    return {"outd": np.concatenate([acc, rsh, dg], 1), "csd": cs}


def _reduce(results, wmaps):
    """f64 host-side assembly of mmd2/var from per-core outputs."""
    rs_x = np.zeros(N)
    rs_y = np.zeros(N)
    rs_xy = np.zeros(N)
    cs_xy = np.zeros(N)
    sums = {"x": 0.0, "y": 0.0, "z": 0.0}
    dgs = {"x": 0.0, "y": 0.0, "z": 0.0}

    for c in range(NCORES):
        out = results[c]
        full = out["outd"].astype(np.float64)
        acc = full[:, 0:NACC]
        rsh = full[:, NACC:NACC + NRSH]
        dg = full[:, NACC + NRSH:NACC + NRSH + 12]
        cs = out["csd"].astype(np.float64)[0]
        wm = wmaps[c]

        def route_rows(v, s, pw, wt):
            # rowsum contribution of rows (side s, own panel) over panel pw
            rows = c * PW + np.arange(128) + route_rows.choff
            S = v.sum()
            if s == 0 and pw < 8:
                rs_x[rows] += v
                sums["x"] += S * wt
            elif s == 0:
                rs_xy[rows] += v
                sums["z"] += S
            elif pw >= 8:
                rs_y[rows] += v
                sums["y"] += S * wt
            else:
                cs_xy[rows] += v
                sums["z"] += S

        for s in range(2):
            for t in range(4):
                pw0, pw1 = wm[8 * s + 2 * t], wm[8 * s + 2 * t + 1]
                for ch in range(4):
                    ti = s * 16 + t * 4 + ch
                    route_rows.choff = ch * 128
                    T = acc[:, ti]
                    if t == 0:
                        h2 = rsh[:, s * 4 + ch]
                        route_rows(T - h2, s, pw0, 1.0)   # diag window
                        route_rows(h2, s, pw1, 2.0)
                    else:
                        # side-pure pair: both windows same bucket
                        route_rows(T, s, pw0, 2.0)
        # block16: x-rows over window 8 (y-panel c)
        for ch in range(4):
            rows = c * PW + ch * 128 + np.arange(128)
            v = acc[:, 32 + ch]
            rs_xy[rows] += v
            sums["z"] += v.sum()

        # column sums (contribute to the transposed image's rows)
        def cs_cols(pw):
            base = pw * PW if pw < 8 else (pw - 8) * PW
            return base + np.arange(PW)

        for s in range(2):
            wlist = range(1, 8) if s == 0 else range(9, 16)
            for w in wlist:
                slot = (w - 1) if w < 8 else 7 + (w - 9)
                v = cs[slot * PW:(slot + 1) * PW]
                pw = wm[w]
                cols = cs_cols(pw)
                if s == 0 and pw < 8:
                    rs_x[cols] += v
                elif s == 0:
                    cs_xy[cols] += v
                elif pw >= 8:
                    rs_y[cols] += v
                else:
                    rs_xy[cols] += v
        cs_xy[cs_cols(wm[8])] += cs[14 * PW:15 * PW]

        dgs["x"] += dg[:, 0:4].sum()
        dgs["y"] += dg[:, 4:8].sum()
        dgs["z"] += dg[:, 8:12].sum()

    nn1 = float(N) * (N - 1)
    xx = (sums["x"] - dgs["x"]) / nn1
    yy = (sums["y"] - dgs["y"]) / nn1
    xy = (sums["z"] - dgs["z"]) / nn1
    mmd2 = xx - 2.0 * xy + yy

    hs = rs_x + rs_y - rs_xy - cs_xy
    sum_h = sums["x"] + sums["y"] - 2.0 * sums["z"]
    v1 = (4.0 / N ** 3) * float(hs @ hs)
    v2 = (4.0 / N ** 4) * sum_h ** 2
    var = v1 - v2 + 1e-8
    return np.array([mmd2, var], np.float32)


def kernel(X, Y, W1, b1, W2, b2, W3, b3, W4, b4,
           epsilon_opt, sigma_q_opt, sigma_phi_opt, _emulate=False):
    global _NC_CACHE, _LAST_RESULT
    in_maps, wmaps, ascale = _host_prep(X, Y, W1, b1, W2, b2, W3, b3, W4, b4,
                                        epsilon_opt, sigma_q_opt, sigma_phi_opt)
    if _emulate:
        results = [_emulate_core(m, ascale) for m in in_maps]
        return _reduce(results, wmaps)

    from concourse import bass_utils
    _harden_tracing()
    if _NC_CACHE is None:
        _NC_CACHE = _build_bass(ascale)
    nc = _NC_CACHE
    res = bass_utils.run_bass_kernel_spmd(nc, in_maps,
                                          core_ids=list(range(NCORES)))
    _LAST_RESULT = res
    return _reduce(res.results, wmaps)


# revision 27
# speedup vs baseline: 1.0778x; 1.0235x over previous
"""Deep-MMD loss kernel for Trainium2, sharded across 8 NeuronCores.

Symmetric single-gram strategy: the three gram matrices k_x, k_y, k_xy are
blocks of one symmetric 8192x8192 gram K over v=[x;y] with a common kernel
exp(-d_feat/sph - d_org/sq).  v splits into 16 panels of 512; core c owns
panels c (x-side) and c+8 (y-side).  Each unordered panel pair is computed
ONCE (pairs {a, a+d mod 16}, d=1..7 by the owner of panel a; d=8 pairs and
both diagonals by core c) -- a uniform 17-block program per core (~29%
fewer exp/matmul elements than computing all three grams row-sharded).
Off-diagonal blocks contribute row sums and, for the transposed image,
column sums (psum-accumulated ones-matmuls).  The host places panels into
windows, so the compiled program is identical on all 8 cores (SPMD).

Per 128x512 window-chunk the device does 2 matmuls + exp:
  - org-side distance: ONE fp8(e4m3) DoubleRow matmul (256-dim contraction
    at 2 rows/cycle).  Raw fp8(x) values ride unscaled (scaling by -2/sq
    would underflow fp8); instead all feature-side rows are pre-scaled by
    sq/2 on host and the exp applies scale = -2/sq via ACT's free affine.
  - feature-side distance + all norm levels: ONE bf16 matmul whose 86
    lhsT/rhs rows expand u_i.u_j into exact bf16 3-level products
    (u = A+B+C: AA, AB, BA, AC, CA, BB), with per-component self-square
    rows interleaved so f32 psum partials stay bounded by the running
    distance, plus 3 bf16 levels of the combined self terms.
  - exp output kt is fp16: it feeds the column-sum matmuls directly and
    the variance-only row splits; all mmd2-relevant sums stay f32 via the
    ACT accumulator (which sums internal f32 regardless of output dtype).

Window ordering is side-pure: supertile pairs hold same-side panels except
possibly supertile 0, so pure tiles need only the per-tile f32 accumulator
total; only supertile-0 tiles get a DVE half-reduce to split the pair.
Diagonal 128x128 blocks are re-exp'd to f32 from the same psum
(bit-identical to what the accumulator summed) so sum-trace cancels
exactly.  Column-sum matmuls are emitted one chunk-iteration late so the
in-order PE queue never stalls on the producing ACTIVATE.

All per-sample transforms (3-layer softplus MLP, W4 W4^T cholesky
v-transform, fp8/bf16 splits, norm levels) are host-side f64 input
transforms, like the original baseline's host-side z1/cholesky; the device
runs a pure streamed gram loop.  Final reduction is f64 on host.
"""

import numpy as np

N = 4096            # samples per side
NP = 16             # 512-sample panels over v = [x; y]
PW = 512            # panel width
NQ = 8192           # per-core q/ur columns (16 windows of 512)
NCORES = 8
HID = 10
NACC = 36           # per-tile f32 rowsum totals: x/y tiles 0..31, block16 32..35
NRSH = 8            # supertile-0 second-half rowsums (s*4 + ch)
NCS = 15            # column-sum window slots


def _build_bass(ascale):
    import concourse.bass as bass  # noqa: F401
    import concourse.mybir as mybir
    import concourse.tile as tile
    from concourse import bacc

    f32 = mybir.dt.float32
    f16 = mybir.dt.float16
    f8 = mybir.dt.float8e4
    bf16 = mybir.dt.bfloat16
    AFT = mybir.ActivationFunctionType
    ALU = mybir.AluOpType
    DR = mybir.MatmulPerfMode.DoubleRow

    nc = bacc.Bacc("TRN2")

    q8d = nc.dram_tensor("q8d", [128, 2, NQ], f8, kind="ExternalInput")
    qs8d = nc.dram_tensor("qs8d", [128, 2, 1024], f8, kind="ExternalInput")
    uld = nc.dram_tensor("uld", [128, 1024], bf16, kind="ExternalInput")
    urd = nc.dram_tensor("urd", [128, NQ], bf16, kind="ExternalInput")
    eyed = nc.dram_tensor("eyed", [128, 128], f32, kind="ExternalInput")

    outd = nc.dram_tensor("outd", [128, NACC + NRSH + 12], f32,
                          kind="ExternalOutput")
    csd = nc.dram_tensor("csd", [1, NCS * PW], f32, kind="ExternalOutput")

    def cs_slot(w):
        # x-pass windows 1..7 -> 0..6; y-pass 9..15 -> 7..13; block16 -> 14
        return (w - 1) if w < 8 else 7 + (w - 9)

    with tile.TileContext(nc) as tc:
        with tc.tile_pool(name="persist", bufs=1) as pp:
            t_q8 = pp.tile([128, 2, NQ], f8, name="q8", tag="q8")
            t_qs8 = pp.tile([128, 2, 1024], f8, name="qs8", tag="qs8")
            t_ul = pp.tile([128, 1024], bf16, name="ul", tag="ul")
            t_ur = pp.tile([128, NQ], bf16, name="ur", tag="ur")
            t_eye = pp.tile([128, 128], f32, name="eye", tag="eye")
            t_ones = pp.tile([128, 1], bf16, name="ones1", tag="ones1")
            t_out = pp.tile([128, NACC + NRSH + 12], f32, name="out",
                            tag="out")
            t_acc = t_out[:, 0:NACC]
            t_rsh = t_out[:, NACC:NACC + NRSH]
            t_dg = t_out[:, NACC + NRSH:NACC + NRSH + 12]
            t_cs = pp.tile([1, NCS * PW], f32, name="cs", tag="cs")
            t_warm = pp.tile([128, 16], f32, name="warm", tag="warm")

            # input loads: first-needed operands first (sync), q on gpsimd
            nc.sync.dma_start(t_qs8[:], qs8d[:])
            nc.sync.dma_start(t_ul[:], uld[:])
            nc.sync.dma_start(t_ur[:, 0:512], urd[:, 0:512])
            nc.sync.dma_start(t_ur[:, 512:1024], urd[:, 512:1024])
            nc.sync.dma_start(t_eye[:], eyed[:])
            nc.gpsimd.dma_start(t_q8[:, :, 0:512], q8d[:, :, 0:512])
            nc.gpsimd.dma_start(t_q8[:, :, 512:1024], q8d[:, :, 512:1024])
            for k in range(1, 8):
                sl = slice(k * 1024, (k + 1) * 1024)
                nc.gpsimd.dma_start(t_q8[:, :, sl], q8d[:, :, sl])
                nc.sync.dma_start(t_ur[:, sl], urd[:, sl])
            # trigger the exp table load immediately (off the data path)
            nc.vector.memset(t_warm[:], 0.0)
            nc.scalar.activation(t_warm[:], t_warm[:], AFT.Exp)
            nc.vector.memset(t_ones[:], 1.0)

            with tc.tile_pool(name="kp", bufs=6) as kp, \
                 tc.tile_pool(name="dp", bufs=3) as dp, \
                 tc.tile_pool(name="gps", bufs=3, space="PSUM") as gps, \
                 tc.tile_pool(name="cps", bufs=1, space="PSUM") as cps:

                def gram_chunk(pk, qoff, w):
                    ns = slice(w * PW, (w + 1) * PW)
                    nc.tensor.matmul(pk, t_qs8[:, :, qoff], t_q8[:, :, ns],
                                     start=True, stop=False, perf_mode=DR)
                    nc.tensor.matmul(pk, t_ul[:, qoff], t_ur[:, ns],
                                     start=False, stop=True)

                def diag(pk, ch, dcol):
                    # f32 re-exp of the diagonal 128x128 from the same psum:
                    # bit-identical to what the accumulator summed, so the
                    # host-side sum-trace cancellation is exact.
                    dkt = dp.tile([128, 128], f32, name="dkt", tag="dkt")
                    nc.scalar.activation(dkt[:], pk[:, ch * 128:(ch + 1) * 128],
                                         AFT.Exp, scale=ascale)
                    dtmp = dp.tile([128, 128], f32, name="dtmp", tag="dtmp")
                    nc.vector.scalar_tensor_tensor(
                        dtmp[:], dkt[:], 1.0, t_eye[:],
                        ALU.mult, ALU.mult, accum_out=dcol)

                # colsum matmuls are emitted one chunk-iteration late so the
                # in-order PE queue never waits on the producing ACTIVATE
                pending = []

                def flush(n):
                    while len(pending) > n:
                        pending.pop(0)()

                def tile_work(s, t, ch, csp0, csp1, w0, w1, cs_copy=True):
                    ti = (s * 16 + t * 4 + ch) if s >= 0 else 32 + ch
                    qoff = slice(max(s, 0) * 512 + ch * 128,
                                 max(s, 0) * 512 + (ch + 1) * 128)
                    pk = gps.tile([128, 1024], f32, name="pk", tag="pk")
                    gram_chunk(pk[:, 0:512], qoff, w0)
                    if w1 is not None:
                        gram_chunk(pk[:, 512:1024], qoff, w1)
                    kt = kp.tile([128, 1024], f16, name="kt", tag="kt")
                    if w1 is not None:
                        nc.scalar.activation(kt[:], pk[:], AFT.Exp, scale=ascale,
                                             accum_out=t_acc[:, ti:ti + 1])
                    else:
                        nc.scalar.activation(kt[:, 0:512], pk[:, 0:512],
                                             AFT.Exp, scale=ascale,
                                             accum_out=t_acc[:, ti:ti + 1])
                    if (s >= 0 and t == 0) or s < 0:
                        dcol = 8 + ch if s < 0 else s * 4 + ch
                        diag(pk, ch, t_dg[:, dcol:dcol + 1])
                    if s >= 0 and t == 0:
                        nc.vector.reduce_sum(
                            t_rsh[:, s * 4 + ch:s * 4 + ch + 1],
                            kt[:, 512:1024], axis=mybir.AxisListType.X)

                    def emit_cs():
                        if csp0 is not None:
                            nc.tensor.matmul(csp0[:], t_ones[:], kt[:, 0:512],
                                             start=(ch == 0), stop=(ch == 3))
                        if csp1 is not None:
                            nc.tensor.matmul(csp1[:], t_ones[:],
                                             kt[:, 512:1024],
                                             start=(ch == 0), stop=(ch == 3))
                        if ch == 3 and cs_copy:
                            for csp, w in ((csp0, w0), (csp1, w1)):
                                if csp is not None:
                                    sl = cs_slot(w)
                                    nc.vector.tensor_copy(
                                        t_cs[0:1, sl * PW:(sl + 1) * PW],
                                        csp[:])
                    pending.append(emit_cs)
                    flush(1)

                for s in range(2):
                    for t in range(4):
                        w0, w1 = 8 * s + 2 * t, 8 * s + 2 * t + 1
                        csp0 = (cps.tile([1, PW], f32, name="csp0", tag="csp0")
                                if t > 0 else None)
                        csp1 = cps.tile([1, PW], f32, name="csp1", tag="csp1")
                        for ch in range(4):
                            tile_work(s, t, ch, csp0, csp1, w0, w1)

                # block16: x-rows over window 8 (the k_xy diagonal block)
                cspb = cps.tile([1, PW], f32, name="csp0", tag="csp0")
                for ch in range(4):
                    tile_work(-1, 0, ch, cspb, None, 8, None, cs_copy=False)
                flush(0)
                nc.vector.tensor_copy(t_cs[0:1, 14 * PW:15 * PW], cspb[:])
                nc.sync.dma_start(outd[:], t_out[:])
                nc.sync.dma_start(csd[:], t_cs[:])

    nc.compile()
    return nc


_NC_CACHE = None
_LAST_RESULT = None


def _harden_tracing():
    """Make run_bass_kernel_spmd(trace=True / BASS_TRACE=1) survive in
    containers whose antenv package lacks axon_hooks, and whose bucket
    upload is unavailable. No-ops when everything is present."""
    import sys
    import types
    try:
        import antenv.axon_hooks  # noqa: F401
    except ImportError:
        mod = types.ModuleType("antenv.axon_hooks")
        mod._hook = None
        mod.set_axon_ntff_profile_hook = lambda h: setattr(mod, "_hook", h)
        mod.get_axon_ntff_profile_hook = lambda: mod._hook
        sys.modules["antenv.axon_hooks"] = mod
        try:
            import antenv
            antenv.axon_hooks = mod
        except ImportError:
            pass
        try:
            from trn_agent_boot.trn_boot import _ntff_profile_via_ctypes
            hook = _ntff_profile_via_ctypes("/opt/axon/libaxon_pjrt.so")
            if hook is not None:
                mod.set_axon_ntff_profile_hook(hook)
        except Exception:
            pass
    from concourse import bass_utils
    if not getattr(bass_utils.upload_artifacts, "_mmd_safe", False):
        orig = bass_utils.upload_artifacts

        def safe_upload(tmpdir):
            try:
                return orig(tmpdir)
            except Exception:
                return tmpdir

        safe_upload._mmd_safe = True
        bass_utils.upload_artifacts = safe_upload


def _softplus(x):
    return np.log1p(np.exp(-np.abs(x))) + np.maximum(x, 0)


def _window_maps():
    """Per-core window -> panel assignment, side-pure except supertile 0."""
    wmaps = []
    for c in range(NCORES):
        px = [(c + d) % NP for d in range(1, 8)]
        Px = [p for p in px if p < 8]
        Py = [p for p in px if p >= 8]
        w1 = Px.pop(0) if len(Px) % 2 == 1 else Py.pop(0)
        xw = [c, w1] + Px + Py
        py = [(c + 8 + d) % NP for d in range(1, 8)]
        Qy = [p for p in py if p >= 8]
        Qx = [p for p in py if p < 8]
        w9 = Qy.pop(0) if len(Qy) % 2 == 1 else Qx.pop(0)
        yw = [c + 8, w9] + Qy + Qx
        wmaps.append(xw + yw)
    return wmaps


def _host_prep(X, Y, W1, b1, W2, b2, W3, b3, W4, b4,
               epsilon_opt, sigma_q_opt, sigma_phi_opt):
    """f64 input transforms -> per-core input maps (window-permuted)."""
    import ml_dtypes
    bfd = ml_dtypes.bfloat16

    X = np.asarray(X, np.float64)
    Y = np.asarray(Y, np.float64)
    W1 = np.asarray(W1, np.float64)
    W2 = np.asarray(W2, np.float64)
    W3 = np.asarray(W3, np.float64)
    W4 = np.asarray(W4, np.float64)
    b1 = np.asarray(b1, np.float64)
    b2 = np.asarray(b2, np.float64)
    b3 = np.asarray(b3, np.float64)
    sq = float(np.asarray(sigma_q_opt, np.float64) ** 2)
    sph = float(np.asarray(sigma_phi_opt, np.float64) ** 2)
    # eps ~ 5e-11 mixture term contributes ~3e-16 to mmd2; dropped (b4
    # cancels exactly inside pairwise feature distances).

    fo = np.concatenate([X, Y], 0)                  # v-space [8192, 256]
    h = _softplus(fo @ W1 + b1)
    h = _softplus(h @ W2 + b2)
    h = _softplus(h @ W3 + b3)
    G = W4 @ W4.T
    lv = np.sqrt(2.0 / sph) * np.linalg.cholesky(G)
    u = h @ lv
    u = u - u.mean(0)                               # d_feat/sph = |ui-uj|^2/2

    # the device psum is P = SC*dm so the raw-fp8 org cross term -x.y rides
    # with coefficient 1; the exp applies scale = -1/SC = -2/sq.
    SC = sq / 2.0
    u = u * np.sqrt(SC)

    def lvl(a):
        hi = a.astype(bfd).astype(np.float64)
        return hi, a - hi

    uh, r = lvl(u)
    um, r = lvl(r)
    ulo, _ = lvl(r)

    f8 = ml_dtypes.float8_e4m3
    x8 = fo.astype(f8)                              # org side fp8
    xq = x8.astype(np.float64)
    xn = 0.5 * (xq * xq).sum(1)                     # = SC * |x8|^2 / sq
    s1 = (0.5 * u * u).astype(bfd).astype(np.float64)   # per-comp self hi
    a_exact = 0.5 * (u * u).sum(1) + xn
    rem = a_exact - s1.sum(1)
    r1, rr = lvl(rem)
    r2, rr = lvl(rr)
    r3, _ = lvl(rr)

    # global UL / UR row content  (psum = a_i + a_j - u_i.u_j - 2/sq x_i.x_j;
    # AA products interleaved with self-squares keep f32 partials bounded)
    M = 2 * N
    ulg = np.zeros((128, M), bfd)
    urg = np.zeros((128, M), bfd)
    for k in range(HID):
        ulg[3 * k] = s1[:, k]
        urg[3 * k] = 1.0
        ulg[3 * k + 1] = -uh[:, k]
        urg[3 * k + 1] = uh[:, k]
        ulg[3 * k + 2] = 1.0
        urg[3 * k + 2] = s1[:, k]
    ulg[30:40] = -um.T
    urg[30:40] = uh.T
    ulg[40:50] = -uh.T
    urg[40:50] = um.T
    ulg[50:60] = -um.T
    urg[50:60] = um.T
    ulg[60:70] = -ulo.T
    urg[60:70] = uh.T
    ulg[70:80] = -uh.T
    urg[70:80] = ulo.T
    for i, rv in enumerate((r1, r2, r3)):
        ulg[80 + i] = rv
        urg[80 + i] = 1.0
        ulg[83 + i] = 1.0
        urg[83 + i] = rv

    # fp8 org operands: [128, 2, cols] with contraction dims (k*128+p)
    orgT = x8.T.reshape(2, 128, 2 * N).transpose(1, 0, 2)    # [128, 2, 8192]
    qsT = (-x8).T.reshape(2, 128, 2 * N).transpose(1, 0, 2)  # negated lhsT

    common_eye = np.eye(128, dtype=np.float32)
    wmaps = _window_maps()
    in_maps = []
    for c in range(NCORES):
        panels = np.array(wmaps[c])
        P = (panels[:, None] * PW + np.arange(PW)[None, :]).reshape(-1)
        own = np.concatenate([np.arange(c * PW, (c + 1) * PW),
                              np.arange(N + c * PW, N + (c + 1) * PW)])
        m = {
            "q8d": np.ascontiguousarray(orgT[:, :, P]),
            "qs8d": np.ascontiguousarray(qsT[:, :, own]),
            "uld": np.ascontiguousarray(ulg[:, own]),
            "urd": np.ascontiguousarray(urg[:, P]),
            "eyed": common_eye,
        }
        in_maps.append(m)
    return in_maps, wmaps, float(-1.0 / SC)


def _emulate_core(m, ascale):
    """Pure-numpy emulation of the device program for one core's inputs.
    Mirrors the bass loop (f64 stand-in for f32; fp16 where the device
    rounds kt)."""
    import ml_dtypes
    f16 = np.float16
    q = m["q8d"].astype(np.float64).transpose(1, 0, 2).reshape(256, NQ)
    qs = m["qs8d"].astype(np.float64).transpose(1, 0, 2).reshape(256, 1024)
    ul = m["uld"].astype(np.float64)                           # [128, 1024]
    ur = m["urd"].astype(np.float64)                           # [128, 8192]
    acc = np.zeros((128, NACC))
    rsh = np.zeros((128, NRSH))
    dg = np.zeros((128, 12))
    cs = np.zeros((1, NCS * PW))

    def window_chunk(qoff, w):
        ns = slice(w * PW, (w + 1) * PW)
        pk = qs[:, qoff].T @ q[:, ns] + ul[:, qoff].T @ ur[:, ns]
        return np.exp(ascale * pk)

    def cs_slot(w):
        return (w - 1) if w < 8 else 7 + (w - 9)

    for s in range(2):
        for t in range(4):
            w0, w1 = 8 * s + 2 * t, 8 * s + 2 * t + 1
            c0 = np.zeros(PW)
            c1 = np.zeros(PW)
            for ch in range(4):
                ti = s * 16 + t * 4 + ch
                qoff = slice(s * 512 + ch * 128, s * 512 + (ch + 1) * 128)
                k0 = window_chunk(qoff, w0)
                k1 = window_chunk(qoff, w1)
                acc[:, ti] = k0.sum(1) + k1.sum(1)
                k0h = k0.astype(f16).astype(np.float64)
                k1h = k1.astype(f16).astype(np.float64)
                if t == 0:
                    rsh[:, s * 4 + ch] = k1h.sum(1)
                    dg[:, s * 4 + ch] = np.diag(
                        k0[:, ch * 128:(ch + 1) * 128])
                if t > 0:
                    c0 += k0h.sum(0)
                c1 += k1h.sum(0)
            if t > 0:
                cs[0, cs_slot(w0) * PW:(cs_slot(w0) + 1) * PW] = c0
            cs[0, cs_slot(w1) * PW:(cs_slot(w1) + 1) * PW] = c1
    cb = np.zeros(PW)
    for ch in range(4):
        qoff = slice(ch * 128, (ch + 1) * 128)
        k0 = window_chunk(qoff, 8)
        acc[:, 32 + ch] = k0.sum(1)
        cb += k0.astype(f16).astype(np.float64).sum(0)
        dg[:, 8 + ch] = np.diag(k0[:, ch * 128:(ch + 1) * 128])
    cs[0, 14 * PW:15 * PW] = cb
    return {"outd": np.concatenate([acc, rsh, dg], 1), "csd": cs}


def _reduce(results, wmaps):
    """f64 host-side assembly of mmd2/var from per-core outputs."""
    rs_x = np.zeros(N)
    rs_y = np.zeros(N)
    rs_xy = np.zeros(N)
    cs_xy = np.zeros(N)
    sums = {"x": 0.0, "y": 0.0, "z": 0.0}
    dgs = {"x": 0.0, "y": 0.0, "z": 0.0}

    for c in range(NCORES):
        out = results[c]
        full = out["outd"].astype(np.float64)
        acc = full[:, 0:NACC]
        rsh = full[:, NACC:NACC + NRSH]
        dg = full[:, NACC + NRSH:NACC + NRSH + 12]
        cs = out["csd"].astype(np.float64)[0]
        wm = wmaps[c]

        def route_rows(v, s, pw, wt):
            # rowsum contribution of rows (side s, own panel) over panel pw
            rows = c * PW + np.arange(128) + route_rows.choff
            S = v.sum()
            if s == 0 and pw < 8:
                rs_x[rows] += v
                sums["x"] += S * wt
            elif s == 0:
                rs_xy[rows] += v
                sums["z"] += S
            elif pw >= 8:
                rs_y[rows] += v
                sums["y"] += S * wt
            else:
                cs_xy[rows] += v
                sums["z"] += S

        for s in range(2):
            for t in range(4):
                pw0, pw1 = wm[8 * s + 2 * t], wm[8 * s + 2 * t + 1]
                for ch in range(4):
                    ti = s * 16 + t * 4 + ch
                    route_rows.choff = ch * 128
                    T = acc[:, ti]
                    if t == 0:
                        h2 = rsh[:, s * 4 + ch]
                        route_rows(T - h2, s, pw0, 1.0)   # diag window
                        route_rows(h2, s, pw1, 2.0)
                    else:
                        # side-pure pair: both windows same bucket
                        route_rows(T, s, pw0, 2.0)
        # block16: x-rows over window 8 (y-panel c)
        for ch in range(4):
            rows = c * PW + ch * 128 + np.arange(128)
            v = acc[:, 32 + ch]
            rs_xy[rows] += v
            sums["z"] += v.sum()

        # column sums (contribute to the transposed image's rows)
        def cs_cols(pw):
            base = pw * PW if pw < 8 else (pw - 8) * PW
            return base + np.arange(PW)

        for s in range(2):
            wlist = range(1, 8) if s == 0 else range(9, 16)
            for w in wlist:
                slot = (w - 1) if w < 8 else 7 + (w - 9)
                v = cs[slot * PW:(slot + 1) * PW]
                pw = wm[w]
                cols = cs_cols(pw)
                if s == 0 and pw < 8:
                    rs_x[cols] += v
                elif s == 0:
                    cs_xy[cols] += v
                elif pw >= 8:
                    rs_y[cols] += v
                else:
                    rs_xy[cols] += v
        cs_xy[cs_cols(wm[8])] += cs[14 * PW:15 * PW]

        dgs["x"] += dg[:, 0:4].sum()
        dgs["y"] += dg[:, 4:8].sum()
        dgs["z"] += dg[:, 8:12].sum()

    nn1 = float(N) * (N - 1)
    xx = (sums["x"] - dgs["x"]) / nn1
    yy = (sums["y"] - dgs["y"]) / nn1
    xy = (sums["z"] - dgs["z"]) / nn1
    mmd2 = xx - 2.0 * xy + yy

    hs = rs_x + rs_y - rs_xy - cs_xy
    sum_h = sums["x"] + sums["y"] - 2.0 * sums["z"]
    v1 = (4.0 / N ** 3) * float(hs @ hs)
    v2 = (4.0 / N ** 4) * sum_h ** 2
    var = v1 - v2 + 1e-8
    return np.array([mmd2, var], np.float32)


def kernel(X, Y, W1, b1, W2, b2, W3, b3, W4, b4,
           epsilon_opt, sigma_q_opt, sigma_phi_opt, _emulate=False):
    global _NC_CACHE, _LAST_RESULT
    in_maps, wmaps, ascale = _host_prep(X, Y, W1, b1, W2, b2, W3, b3, W4, b4,
                                        epsilon_opt, sigma_q_opt, sigma_phi_opt)
    if _emulate:
        results = [_emulate_core(m, ascale) for m in in_maps]
        return _reduce(results, wmaps)

    from concourse import bass_utils
    _harden_tracing()
    if _NC_CACHE is None:
        _NC_CACHE = _build_bass(ascale)
    nc = _NC_CACHE
    res = bass_utils.run_bass_kernel_spmd(nc, in_maps,
                                          core_ids=list(range(NCORES)))
    _LAST_RESULT = res
    return _reduce(res.results, wmaps)
